# revision 24
# baseline (speedup 1.0000x reference)
"""Trainium2 Bass kernel for a 3-scale YOLO-face Detect head (nms_detection).

Sharding: data-parallel over batch (16 images -> 2 per core x 8 cores).

Per-core plan (fp8/bf16 matmuls, 32-slot psum groups, chunked stores):
  Pixels of each (image, scale) are split into chunks of Q*S pixels laid out
  so partition q owns the S *consecutive* pixels chunk_base + q*S + [0, S).
  x0/x1 are cast fp32->fp8e3m4 on host (halves their HBM traffic; the 4-bit
  mantissa keeps the C=128/256 dot products within tolerance), x2 stays
  bf16; matmuls run with fp8 lhsT against bf16 weights.

  A chunk is processed in groups of up to 32 psum slots (4 banks); each
  pixel-column gets a 64-wide psum slot (57 channels + 7 pad) so a group is
  a uniform [q, nsl, 64] view and each decode op covers the whole group:
    - kc fp8/bf16 matmuls per slot: lhsT = x[:, q*S + sl] (pixel-strided
      stationary), rhs = [C128, 57] weight chunk; start=True on the first
      write into each psum BANK.
    - ONE K=1 matmul (ones x 32x-tiled 64-padded bias row) adds the conv
      bias to ALL 57 channels of every slot in the group.
    - ACT sigmoids o 0:5 and cls 17:19 into the output tile; DVE adds the
      anchor-independent grid table (broadcast over anchors) to the
      landmarks (bias already in psum via the wide bias matmul).
  Chunk-wide: DVE squares the wh sigmoid, builds xy via
  scalar_tensor_tensor with an anchor-broadcast grid table, then ONE dma
  (issued from the gpsimd queue to keep the hot engines free) stores the
  [Q, 3*S*19] tile with S*38-byte contiguous segments per (q, anchor).

  Grid tables are anchor-free: btl12 [Q, nch*S*12] = stride*(gx,gy) x6 for
  the landmarks, btab [Q, nch*S*2] = btl12 pair - stride/2 for xy; both are
  generated on-chip from tiny [Q, S] gx/gy seed constants.
"""

import sys

for _p in ("/opt/trn_rl_repo", "/root/.axon_site/_ro/trn_rl_repo"):
    if _p not in sys.path:
        sys.path.append(_p)

from contextlib import ExitStack

import ml_dtypes
import numpy as np

import concourse.bass as bass
import concourse.tile as tile
from concourse import mybir
from concourse.bass_utils import run_bass_kernel_spmd

F32 = mybir.dt.float32
BF16 = mybir.dt.bfloat16
F8E3 = mybir.dt.float8e3  # e3m4: 4 mantissa bits, range +-15.5
AF = mybir.ActivationFunctionType
OP = mybir.AluOpType

N_CORES = 8
BS = 16
B_LOC = BS // N_CORES  # 2 images per core

NA = 3
NO = 19
NCH = NA * NO  # 57

STRIDES = (8.0, 16.0, 32.0)
ANCHORS = np.array(
    [[10, 13, 16, 30, 33, 23],
     [30, 61, 62, 45, 59, 119],
     [116, 90, 156, 198, 373, 326]],
    dtype=np.float32,
).reshape(3, NA, 2)

# per scale: channels, k-chunks, image size, partitions, px/partition/chunk,
# chunks per image
SCALES = [
    dict(C=128, kc=1, ny=160, nx=160, Q=128, S=40, nch=5),
    dict(C=256, kc=2, ny=80, nx=80, Q=128, S=25, nch=2),
    dict(C=512, kc=4, ny=40, nx=40, Q=100, S=16, nch=1),
]
for s in SCALES:
    s["npix"] = s["ny"] * s["nx"]
    assert s["nch"] * s["Q"] * s["S"] == s["npix"]

GSL = 32  # psum slots per group (4 banks of 8x64-wide slots)

OUT_BASE = [0, 3 * SCALES[0]["npix"], 3 * (SCALES[0]["npix"] + SCALES[1]["npix"])]
TOT_ROWS = 3 * sum(s["npix"] for s in SCALES)  # 100800

# cblob column offsets: a4 tables + per-scale [Q, S] gx/gy seed tables
# (gx/gy of pixel q*S+s; the chunk offset ch*Q*S only shifts gy, by Q*S/nx
# per chunk, since nx divides Q*S for every scale)
A4_OFF = 0
GX_OFF = [18, 98, 148]
GY_OFF = [58, 123, 164]
CB_W = 180

# wpack layout: seven [128, 57] bf16 wT chunks (s0k0, s1k0, s1k1, s2k0..3)
# in cols [0, 399); ones rows for the K=1 bias matmuls in cols [399, 527)
# on partitions 0/32/64; per-(scale, anchor) 12-wide lm bias rows
# (anchor-scaled) in cols [527, 635) on all partitions.
W_ONES = 399
BLM_OFF = 527
WP_W = 635

# psum column packing: o-major with the sigmoid channels first:
# cols p -> channel (o, a) where o runs {0..4, 17, 18, 5..16}, a minor
O_ORDER = list(range(5)) + [17, 18] + list(range(5, 17))
PERM = np.array(
    [a * NO + o for o in O_ORDER for a in range(NA)], dtype=np.int64
)


def _lm_factor(si):
    """57-vector: anchor scale for landmark channels, 1 elsewhere."""
    fac = np.ones(NCH, dtype=np.float32)
    for a in range(NA):
        for o in range(5, 17):
            fac[a * NO + o] = ANCHORS[si, a, (o - 5) % 2]
    return fac


def _a4tab(si):
    """[128, 6] table of 4*anchor for the wh channels, replicated on partitions."""
    v = (4.0 * ANCHORS[si]).reshape(1, NA * 2).astype(ml_dtypes.bfloat16)
    return np.broadcast_to(v, (128, NA * 2)).copy()


def _build_program():
    import os
    dbg_scales = [int(c) for c in os.environ.get("K_SCALES", "012")]
    dbg_imgs = int(os.environ.get("K_IMGS", str(B_LOC)))

    nc = bass.Bass("TRN2", target_bir_lowering=False, num_devices=N_CORES)

    x_in = [
        nc.dram_tensor("x0", [B_LOC, 128, 160, 160], F8E3, kind="ExternalInput"),
        nc.dram_tensor("x1", [B_LOC, 256, 80, 80], F8E3, kind="ExternalInput"),
        nc.dram_tensor("x2", [B_LOC, 512, 40, 40], BF16, kind="ExternalInput"),
    ]
    wpack_in = nc.dram_tensor("wpack", [128, WP_W], BF16, kind="ExternalInput")
    # 32x-tiled 64-padded per-scale bias rows for the wide bias matmul
    brow_in = nc.dram_tensor("brow", [3, GSL * 64], BF16, kind="ExternalInput")
    out = nc.dram_tensor("out", [B_LOC, TOT_ROWS, NO], BF16, kind="ExternalOutput")

    # Compile-time constants: gx/gy seed tables (fp32) + a4 tables (bf16).
    cblob = np.zeros((128, CB_W), dtype=np.float32)
    a4blob = np.zeros((128, 18), dtype=ml_dtypes.bfloat16)
    for i in range(3):
        a4blob[:, 6 * i:6 * i + 6] = _a4tab(i)
        s = SCALES[i]
        Q, S, nx = s["Q"], s["S"], s["nx"]
        pix = np.arange(Q)[:, None] * S + np.arange(S)[None, :]
        cblob[:Q, GX_OFF[i]:GX_OFF[i] + S] = (pix % nx).astype(np.float32)
        cblob[:Q, GY_OFF[i]:GY_OFF[i] + S] = (pix // nx).astype(np.float32)
    cblob_c = nc.inline_tensor(cblob, name="cblob")
    a4blob_c = nc.inline_tensor(a4blob, name="a4blob")

    with tile.TileContext(nc) as tc, ExitStack() as ctx:
        const_pool = ctx.enter_context(tc.tile_pool(name="consts", bufs=1))
        x0_pool = ctx.enter_context(tc.tile_pool(name="x0p", bufs=8))
        x1_pool = ctx.enter_context(tc.tile_pool(name="x1p", bufs=4))
        x2_pool = ctx.enter_context(tc.tile_pool(name="x2p", bufs=2))
        ps_pool = ctx.enter_context(tc.tile_pool(name="ps", bufs=2, space="PSUM"))
        o_pool = ctx.enter_context(tc.tile_pool(name="outp", bufs=6))

        # ---- persistent constants / weights ------------------------------
        wp = const_pool.tile([128, WP_W], BF16, tag="wpack")
        nc.scalar.dma_start(wp[:], wpack_in.ap()[:, :])
        cb = const_pool.tile([128, CB_W], F32, tag="cblob")
        nc.scalar.dma_start(cb[:], cblob_c.ap()[:, :])
        a4t = const_pool.tile([128, 18], BF16, tag="a4blob")
        nc.scalar.dma_start(a4t[:], a4blob_c.ap()[:, :])
        brt = const_pool.tile([128, GSL * 64], BF16, tag="brow")
        for i in range(3):
            nc.scalar.dma_start(
                brt[32 * i:32 * i + 1, :], brow_in.ap()[i:i + 1, :]
            )

        # ---- grid tables, generated on-chip ------------------------------
        # btl12[q, c, s, 0:12] = stride*(gx, gy) repeated 6x (lm grid)
        # btl3 [q, a, c, s, 0:12] = btl12 + lm bias row for anchor a
        # btab [q, c, s, 0:2]  = stride*(gx, gy) - stride/2 (xy grid)
        btl3_sb = []
        btab_sb = []
        for i in range(3):
            s = SCALES[i]
            Q, S, nch, nx = s["Q"], s["S"], s["nch"], s["nx"]
            stride = STRIDES[i]
            CS = nch * S
            gxq = cb[:Q, GX_OFF[i]:GX_OFF[i] + S]
            gyq = cb[:Q, GY_OFF[i]:GY_OFF[i] + S]
            # gy(ch, q, s) = (Q*S/nx)*ch + gyq[q, s]
            gyt = const_pool.tile([128, CS], F32, tag=f"gy{i}")
            gy3 = gyt[:Q, :CS].rearrange("q (c s) -> q c s", c=nch, s=S)
            nc.gpsimd.iota(
                gy3, [[Q * S // nx, nch], [0, S]], base=0,
                channel_multiplier=0,
                allow_small_or_imprecise_dtypes=True,
            )
            nc.vector.tensor_tensor(
                gy3, gy3,
                gyq.unsqueeze(1).broadcast_to((Q, nch, S)), op=OP.add,
            )
            bt = const_pool.tile([128, CS * 12], BF16, tag=f"btl12{i}")
            btv = bt[:Q].rearrange("q (c s o) -> q c s o", c=nch, s=S, o=12)
            nc.scalar.mul(
                btv[:, :, :, 0:12:2],
                gxq.unsqueeze(1).unsqueeze(3).broadcast_to((Q, nch, S, 6)),
                stride,
            )
            nc.scalar.mul(
                btv[:, :, :, 1:12:2],
                gy3.unsqueeze(3).broadcast_to((Q, nch, S, 6)),
                stride,
            )
            ba = const_pool.tile([128, CS * 2], BF16, tag=f"btab{i}")
            bav = ba[:Q].rearrange("q (c s o) -> q c s o", c=nch, s=S, o=2)
            nc.vector.tensor_scalar(
                bav, btv[:, :, :, 0:2], -0.5 * stride, None, op0=OP.add
            )
            bt3 = const_pool.tile([128, NA * CS * 12], BF16, tag=f"btl3{i}")
            bt3v = bt3[:Q].rearrange(
                "q (a c s o) -> q a c s o", a=NA, c=nch, s=S, o=12
            )
            for a in range(NA):
                blr = wp[:Q, BLM_OFF + 36 * i + 12 * a:BLM_OFF + 36 * i + 12 * a + 12]
                nc.vector.tensor_tensor(
                    bt3v[:, a], btv,
                    blr.unsqueeze(1).unsqueeze(2)
                    .broadcast_to((Q, nch, S, 12)),
                    op=OP.add,
                )
            btl3_sb.append(bt3v)
            btab_sb.append(bav)

        wt_sb = []  # [scale][kc] -> [128, 57] AP
        off = 0
        for i in range(3):
            chunks = []
            for k in range(SCALES[i]["kc"]):
                chunks.append(wp[:, off:off + NCH])
                off += NCH
            wt_sb.append(chunks)
        b8_sb = [
            brt[32 * i:32 * i + 1, :].rearrange("p (sl c) -> p sl c", c=64)
            for i in range(3)
        ]
        a4_sb = [a4t[:, 6 * i:6 * i + 6] for i in range(3)]
        ones_sb = [wp[32 * i:32 * i + 1, W_ONES:W_ONES + 128] for i in range(3)]

        out_ap = out.ap()
        pending = []  # deferred per-chunk fixup+store emitters

        def do_chunk(si, b, x_aps, ch):
            """Emit one Q*S-pixel chunk: 32-slot psum groups + decode + store.

            x_aps: per-K-chunk [128, Q, S] SBUF APs (c, q, s).
            """
            s = SCALES[si]
            Q, S, kc = s["Q"], s["S"], s["kc"]
            stride = STRIDES[si]

            ot = o_pool.tile([128, 3 * 40 * NO], BF16)
            otv = ot[:Q, : NA * S * NO]
            o_v = otv.rearrange("q (a s o) -> q a s o", a=NA, s=S, o=NO)

            for g0 in range(0, S, GSL):
                gsl = min(GSL, S - g0)
                ps = ps_pool.tile([128, GSL * 64], F32)
                psv = ps[:Q]
                for sl in range(gsl):
                    for k in range(kc):
                        nc.tensor.matmul(
                            psv[:, sl * 64:sl * 64 + NCH],
                            lhsT=x_aps[k][:, :, g0 + sl],
                            rhs=wt_sb[si][k],
                            start=(sl * 64 % 512 == 0 and k == 0),
                            stop=False,
                        )
                p_sl = psv[:, : gsl * 64].rearrange("q (sl c) -> q sl c", c=64)
                # conv bias for the 21 sigmoid/cls channels of every slot
                # (lm bias rides the btl3 table); one matmul per bank
                # (matmul N is capped at one psum bank)
                for s0_ in range(0, gsl, 8):
                    s1_ = min(gsl, s0_ + 8)
                    nc.tensor.matmul(
                        p_sl[:, s0_:s1_, 0:21],
                        lhsT=ones_sb[si][:, :Q],
                        rhs=b8_sb[si][:, s0_:s1_, 0:21],
                        start=False,
                        stop=(s1_ == gsl),
                    )
                # sigmoid of o 0:5 (xy/wh/conf) straight into the output
                # tile; xy/wh are fixed up in place chunk-wide below
                nc.scalar.activation(
                    o_v[:, :, g0:g0 + gsl, 0:5],
                    p_sl[:, :, 0:15].rearrange(
                        "q sl (o a) -> q a sl o", o=5, a=NA
                    ),
                    AF.Sigmoid,
                )
                # cls: sigmoid straight into the output tile
                nc.scalar.activation(
                    o_v[:, :, g0:g0 + gsl, 17:19],
                    p_sl[:, :, 15:21].rearrange(
                        "q sl (o a) -> q a sl o", o=2, a=NA
                    ),
                    AF.Sigmoid,
                )
                # lm = p (anchor-scaled in weights) + grid + anchor bias
                btl = btl3_sb[si][:, :, ch, g0:g0 + gsl, :]
                nc.vector.tensor_tensor(
                    o_v[:, :, g0:g0 + gsl, 5:17],
                    p_sl[:, :, 21:NCH].rearrange(
                        "q sl (o a) -> q a sl o", o=12, a=NA
                    ),
                    btl, op=OP.add,
                )

            # ---- chunk-wide fixups + store, deferred one chunk -----------
            # (emitted after the NEXT chunk's groups, so the DVE queue ahead
            # of each psum-freeing lm add holds only lm adds and the PE
            # never stalls on a psum buffer)
            def finish(si=si, b=b, ch=ch, o_v=o_v, Q=Q, S=S, stride=stride):
                s = SCALES[si]
                # xy = sig*(2*stride) + btab, anchor-broadcast grid table
                btc = (
                    btab_sb[si][:, ch]
                    .unsqueeze(1)
                    .broadcast_to((Q, NA, S, 2))
                )
                nc.vector.scalar_tensor_tensor(
                    o_v[:, :, :, 0:2], o_v[:, :, :, 0:2], 2.0 * stride, btc,
                    op0=OP.mult, op1=OP.add,
                )
                # wh = square(sig) * 4*anchor: square on ACT, mult on DVE
                nc.scalar.square(o_v[:, :, :, 2:4], o_v[:, :, :, 2:4])
                a4 = (
                    a4_sb[si][:Q, :]
                    .rearrange("q (a o) -> q a o", a=NA, o=2)
                    .unsqueeze(2)
                    .broadcast_to((Q, NA, S, 2))
                )
                nc.vector.tensor_tensor(
                    o_v[:, :, :, 2:4], o_v[:, :, :, 2:4], a4, op=OP.mult
                )
                # one store per chunk: S*38B contiguous per (q, anchor)
                dst = (
                    out_ap[b, OUT_BASE[si]:OUT_BASE[si] + NA * s["npix"], :]
                    .rearrange(
                        "(a ch q s) o -> ch q a s o",
                        a=NA, ch=s["nch"], q=Q, s=S,
                    )
                )
                nc.gpsimd.dma_start(dst[ch], o_v)

            pending.append(finish)
            while len(pending) > 1:
                pending.pop(0)()

        for b in range(dbg_imgs):
            if 0 in dbg_scales:
                s = SCALES[0]
                x0_flat = x_in[0].ap()[b].rearrange("c h w -> c (h w)")
                cpx = s["Q"] * s["S"]
                for ch in range(s["nch"]):
                    xt = x0_pool.tile([128, cpx], F8E3)
                    nc.sync.dma_start(
                        xt[:], x0_flat[:, ch * cpx:(ch + 1) * cpx]
                    )
                    x4 = xt[:].rearrange("c (s q) -> c q s", s=s["S"], q=s["Q"])
                    do_chunk(0, b, [x4], ch)

            if 1 in dbg_scales:
                s = SCALES[1]
                kc = s["kc"]
                x1_k = x_in[1].ap()[b].rearrange(
                    "(k c) h w -> c k (h w)", k=kc
                )
                cpx = s["Q"] * s["S"]
                for ch in range(s["nch"]):
                    t = x1_pool.tile([128, kc * cpx], F8E3)
                    nc.sync.dma_start(
                        t[:].rearrange("c (k p) -> c k p", k=kc),
                        x1_k[:, :, ch * cpx:(ch + 1) * cpx],
                    )
                    x5 = t[:].rearrange(
                        "c (k s q) -> c k q s", k=kc, s=s["S"], q=s["Q"]
                    )
                    do_chunk(1, b, [x5[:, k] for k in range(kc)], ch)

            if 2 in dbg_scales:
                s = SCALES[2]
                kc = s["kc"]
                x2_k = x_in[2].ap()[b].rearrange(
                    "(k c) h w -> c k (h w)", k=kc
                )
                t = x2_pool.tile([128, kc * s["npix"]], BF16)
                nc.sync.dma_start(
                    t[:].rearrange("c (k p) -> c k p", k=kc), x2_k
                )
                x5 = t[:].rearrange(
                    "c (k s q) -> c k q s", k=kc, s=s["S"], q=s["Q"]
                )
                do_chunk(2, b, [x5[:, k] for k in range(kc)], 0)

        while pending:
            pending.pop(0)()

    return nc


# Instruction types walrus accepts multiple sync-waits on.  Empirically none:
# even the kernel-tail Drain gets rejected with >1 wait.
_MULTI_WAIT_OK = set()


def _legalize_waits(nc):
    """Spill extra sync waits onto single-wait NoOps.

    walrus's per-instruction ISA structs hold a limited number of sync wait
    commands (a Matmult's LDWEIGHTS holds exactly one), and Tile's semaphore
    assignment doesn't know that.  Rewrite the scheduled program so every
    instruction carries at most one wait; the rest go to same-engine NoOps
    placed immediately before it (same blocking semantics).
    """
    f = nc.m.functions[0]
    for blk in f.blocks:
        insts = blk.instructions
        out = []
        changed = False
        for inst in insts:
            si = inst.sync_info
            if (
                si is not None
                and len(si.on_wait) > 1
                and type(inst).__name__ not in _MULTI_WAIT_OK
            ):
                waits = list(si.on_wait)
                for w in waits[:-1]:
                    nop = mybir.InstNoOp(
                        name=nc.get_next_instruction_name(),
                        engine=inst.engine,
                        ins=[],
                        outs=[],
                        sync_info=mybir.SyncInfo(on_wait=[w], on_update=[]),
                    )
                    out.append(nop)
                inst.sync_info = mybir.SyncInfo(
                    on_wait=[waits[-1]], on_update=list(si.on_update)
                )
                changed = True
            out.append(inst)
        if changed:
            blk.instructions = out


_NC_CACHE = None
_LEGALIZED = False


def _get_program(legalize=False):
    """Build (and cache) the Bass program.

    legalize=True applies the walrus wait-limit rewrite; the CoreSim can only
    run the raw (unlegalized) program, so this is done lazily for HW runs.
    """
    global _NC_CACHE, _LEGALIZED
    if _NC_CACHE is None:
        _NC_CACHE = _build_program()
    if legalize and not _LEGALIZED:
        _legalize_waits(_NC_CACHE)
        _LEGALIZED = True
    return _NC_CACHE


def _prep_inputs(x0, x1, x2, w0, w1, w2, b0, b1, b2):
    ws = (w0, w1, w2)
    bs = (b0, b1, b2)
    wpack = np.zeros((128, WP_W), dtype=ml_dtypes.bfloat16)
    brow = np.zeros((3, GSL * 64), dtype=ml_dtypes.bfloat16)
    off = 0
    for i in range(3):
        fac = _lm_factor(i)
        wt = (np.asarray(ws[i], np.float32).T * fac[None, :]).astype(np.float32)
        wt = wt[:, PERM]
        for k in range(SCALES[i]["kc"]):
            wpack[:, off:off + NCH] = wt[k * 128:(k + 1) * 128]
            off += NCH
        wpack[32 * i, W_ONES:W_ONES + 128] = 1.0
        bfac = np.asarray(bs[i], np.float32) * fac
        b57 = bfac[PERM]
        slot = np.concatenate([b57, np.zeros(64 - NCH, np.float32)])
        brow[i] = np.tile(slot, GSL)
        blm = np.stack(
            [bfac[a * NO + 5:a * NO + 17] for a in range(NA)]
        ).reshape(-1)
        wpack[:, BLM_OFF + 36 * i:BLM_OFF + 36 * i + 36] = blm[None, :]
    x_np_dt = (ml_dtypes.float8_e3m4, ml_dtypes.float8_e3m4, ml_dtypes.bfloat16)
    xs = []
    for i, x in enumerate((x0, x1, x2)):
        sc = SCALES[i]
        v = np.asarray(x, np.float32).astype(x_np_dt[i])
        B, C = v.shape[0], v.shape[1]
        # (q, s) -> (s, q) within each chunk so matmul weight columns are
        # contiguous in SBUF (enables fast weight load on the PE)
        v = v.reshape(B, C, sc["nch"], sc["Q"], sc["S"])
        v = np.ascontiguousarray(v.transpose(0, 1, 2, 4, 3))
        xs.append(v.reshape(B, C, x.shape[2], x.shape[3]))
    in_maps = []
    for c in range(N_CORES):
        m = {"wpack": wpack, "brow": brow}
        for i, x in enumerate(xs):
            m[f"x{i}"] = np.ascontiguousarray(x[c * B_LOC:(c + 1) * B_LOC])
        in_maps.append(m)
    return in_maps


def _run(inputs, trace=False):
    nc = _get_program(legalize=True)
    in_maps = _prep_inputs(**inputs)
    res = run_bass_kernel_spmd(nc, in_maps, list(range(N_CORES)), trace=trace)
    out = np.concatenate([r["out"] for r in res.results], axis=0)
    return out.astype(np.float32), res


def kernel(x0, x1, x2, w0, w1, w2, b0, b1, b2):
    out, _ = _run(
        dict(x0=x0, x1=x1, x2=x2, w0=w0, w1=w1, w2=w2, b0=b0, b1=b1, b2=b2)
    )
    return out


# revision 31
# speedup vs baseline: 1.1906x; 1.1906x over previous
"""Trainium2 Bass kernel for a 3-scale YOLO-face Detect head (nms_detection).

Sharding: data-parallel over batch (16 images -> 2 per core x 8 cores).

Per-core plan (fp8/bf16 matmuls, 32-slot psum groups, chunked stores):
  Pixels of each (image, scale) are split into chunks of Q*S pixels laid out
  so partition q owns the S *consecutive* pixels chunk_base + q*S + [0, S).
  x0/x1 are cast fp32->fp8e3m4 on host (halves their HBM traffic; the 4-bit
  mantissa keeps the C=128/256 dot products within tolerance), x2 stays
  bf16; matmuls run with fp8 lhsT against bf16 weights.

  A chunk is processed in groups of up to 32 psum slots (4 banks); each
  pixel-column gets a 64-wide psum slot (57 channels + 7 pad) so a group is
  a uniform [q, nsl, 64] view and each decode op covers the whole group:
    - kc fp8/bf16 matmuls per slot: lhsT = x[:, q*S + sl] (pixel-strided
      stationary), rhs = [C128, 57] weight chunk; start=True on the first
      write into each psum BANK.
    - ONE K=1 matmul (ones x 32x-tiled 64-padded bias row) adds the conv
      bias to ALL 57 channels of every slot in the group.
    - ACT sigmoids o 0:5 and cls 17:19 into the output tile; DVE adds the
      anchor-independent grid table (broadcast over anchors) to the
      landmarks (bias already in psum via the wide bias matmul).
  Chunk-wide: DVE squares the wh sigmoid, builds xy via
  scalar_tensor_tensor with an anchor-broadcast grid table, then ONE dma
  (issued from the gpsimd queue to keep the hot engines free) stores the
  [Q, 3*S*19] tile with S*38-byte contiguous segments per (q, anchor).

  Grid tables are anchor-free: btl12 [Q, nch*S*12] = stride*(gx,gy) x6 for
  the landmarks, btab [Q, nch*S*2] = btl12 pair - stride/2 for xy; both are
  generated on-chip from tiny [Q, S] gx/gy seed constants.
"""

import sys

for _p in ("/opt/trn_rl_repo", "/root/.axon_site/_ro/trn_rl_repo"):
    if _p not in sys.path:
        sys.path.append(_p)

from contextlib import ExitStack

import ml_dtypes
import numpy as np

import concourse.bass as bass
import concourse.tile as tile
from concourse import mybir
from concourse.bass_utils import run_bass_kernel_spmd

F32 = mybir.dt.float32
BF16 = mybir.dt.bfloat16
F8E3 = mybir.dt.float8e3  # e3m4: 4 mantissa bits, range +-15.5
AF = mybir.ActivationFunctionType
OP = mybir.AluOpType

N_CORES = 8
BS = 16
B_LOC = BS // N_CORES  # 2 images per core

NA = 3
NO = 19
NCH = NA * NO  # 57

STRIDES = (8.0, 16.0, 32.0)
ANCHORS = np.array(
    [[10, 13, 16, 30, 33, 23],
     [30, 61, 62, 45, 59, 119],
     [116, 90, 156, 198, 373, 326]],
    dtype=np.float32,
).reshape(3, NA, 2)

# per scale: channels, k-chunks, image size, partitions, px/partition/chunk,
# chunks per image
SCALES = [
    dict(C=128, kc=1, ny=160, nx=160, Q=128, S=40, nch=5),
    dict(C=256, kc=2, ny=80, nx=80, Q=128, S=25, nch=2),
    dict(C=512, kc=4, ny=40, nx=40, Q=100, S=16, nch=1),
]
for s in SCALES:
    s["npix"] = s["ny"] * s["nx"]
    assert s["nch"] * s["Q"] * s["S"] == s["npix"]

GSL = 16  # psum slots per group (2 banks of 8x64-wide slots)

OUT_BASE = [0, 3 * SCALES[0]["npix"], 3 * (SCALES[0]["npix"] + SCALES[1]["npix"])]
TOT_ROWS = 3 * sum(s["npix"] for s in SCALES)  # 100800

# cblob column offsets: a4 tables + per-scale [Q, S] gx/gy seed tables
# (gx/gy of pixel q*S+s; the chunk offset ch*Q*S only shifts gy, by Q*S/nx
# per chunk, since nx divides Q*S for every scale)
A4_OFF = 0
GX_OFF = [18, 98, 148]
GY_OFF = [58, 123, 164]
CB_W = 180

# wpack layout: seven [128, 57] bf16 wT chunks (s0k0, s1k0, s1k1, s2k0..3)
# in cols [0, 399); ones rows for the K=1 bias matmuls in cols [399, 527)
# on partitions 0/32/64; per-(scale, anchor) 12-wide lm bias rows
# (anchor-scaled) in cols [527, 635) on all partitions.
W_ONES = 399
BLM_OFF = 527
WP_W = 635

# psum column packing: o-major with the sigmoid channels first:
# cols p -> channel (o, a) where o runs {0..4, 17, 18, 5..16}, a minor
O_ORDER = list(range(5)) + [17, 18] + list(range(5, 17))
PERM = np.array(
    [a * NO + o for o in O_ORDER for a in range(NA)], dtype=np.int64
)


def _lm_factor(si):
    """57-vector: anchor scale for landmark channels, 1 elsewhere."""
    fac = np.ones(NCH, dtype=np.float32)
    for a in range(NA):
        for o in range(5, 17):
            fac[a * NO + o] = ANCHORS[si, a, (o - 5) % 2]
    return fac


def _a4tab(si):
    """[128, 6] table of 4*anchor for the wh channels, replicated on partitions."""
    v = (4.0 * ANCHORS[si]).reshape(1, NA * 2).astype(ml_dtypes.bfloat16)
    return np.broadcast_to(v, (128, NA * 2)).copy()


def _build_program():
    import os
    dbg_scales = [int(c) for c in os.environ.get("K_SCALES", "012")]
    dbg_imgs = int(os.environ.get("K_IMGS", str(B_LOC)))

    nc = bass.Bass("TRN2", target_bir_lowering=False, num_devices=N_CORES)

    x_in = [
        nc.dram_tensor("x0", [B_LOC, 128, 160, 160], F8E3, kind="ExternalInput"),
        nc.dram_tensor("x1", [B_LOC, 256, 80, 80], F8E3, kind="ExternalInput"),
        nc.dram_tensor("x2", [B_LOC, 512, 40, 40], BF16, kind="ExternalInput"),
    ]
    wpack_in = nc.dram_tensor("wpack", [128, WP_W], BF16, kind="ExternalInput")
    # tiled 64-padded per-scale bias rows for the per-group bias matmul
    brow_in = nc.dram_tensor("brow", [3, 32 * 64], BF16, kind="ExternalInput")
    out = nc.dram_tensor("out", [B_LOC, TOT_ROWS, NO], BF16, kind="ExternalOutput")

    # Compile-time constants: gx/gy seed tables (fp32) + a4 tables (bf16).
    cblob = np.zeros((128, CB_W), dtype=np.float32)
    a4blob = np.zeros((128, 18), dtype=ml_dtypes.bfloat16)
    for i in range(3):
        a4blob[:, 6 * i:6 * i + 6] = _a4tab(i)
        s = SCALES[i]
        Q, S, nx = s["Q"], s["S"], s["nx"]
        pix = np.arange(Q)[:, None] * S + np.arange(S)[None, :]
        cblob[:Q, GX_OFF[i]:GX_OFF[i] + S] = (pix % nx).astype(np.float32)
        cblob[:Q, GY_OFF[i]:GY_OFF[i] + S] = (pix // nx).astype(np.float32)
    cblob_c = nc.inline_tensor(cblob, name="cblob")
    a4blob_c = nc.inline_tensor(a4blob, name="a4blob")

    with tile.TileContext(nc) as tc, ExitStack() as ctx:
        const_pool = ctx.enter_context(tc.tile_pool(name="consts", bufs=1))
        x0_pool = ctx.enter_context(tc.tile_pool(name="x0p", bufs=8))
        x1_pool = ctx.enter_context(tc.tile_pool(name="x1p", bufs=4))
        x2_pool = ctx.enter_context(tc.tile_pool(name="x2p", bufs=2))
        ps_pool = ctx.enter_context(tc.tile_pool(name="ps", bufs=4, space="PSUM"))
        o_pool = ctx.enter_context(tc.tile_pool(name="outp", bufs=6))

        # ---- persistent constants / weights ------------------------------
        wp = const_pool.tile([128, WP_W], BF16, tag="wpack")
        nc.scalar.dma_start(wp[:], wpack_in.ap()[:, :])
        cb = const_pool.tile([128, CB_W], F32, tag="cblob")
        nc.scalar.dma_start(cb[:], cblob_c.ap()[:, :])
        a4t = const_pool.tile([128, 18], BF16, tag="a4blob")
        nc.scalar.dma_start(a4t[:], a4blob_c.ap()[:, :])
        brt = const_pool.tile([128, 32 * 64], BF16, tag="brow")
        for i in range(3):
            nc.scalar.dma_start(
                brt[32 * i:32 * i + 1, :], brow_in.ap()[i:i + 1, :]
            )

        # ---- grid tables, generated on-chip ------------------------------
        # btl12[q, c, s, 0:12] = stride*(gx, gy) repeated 6x (lm grid)
        # btl3 [q, a, c, s, 0:12] = btl12 + lm bias row for anchor a
        # btab [q, c, s, 0:2]  = stride*(gx, gy) - stride/2 (xy grid)
        btl3_sb = []
        btab_sb = []
        for i in range(3):
            s = SCALES[i]
            Q, S, nch, nx = s["Q"], s["S"], s["nch"], s["nx"]
            stride = STRIDES[i]
            CS = nch * S
            gxq = cb[:Q, GX_OFF[i]:GX_OFF[i] + S]
            gyq = cb[:Q, GY_OFF[i]:GY_OFF[i] + S]
            # gy(ch, q, s) = (Q*S/nx)*ch + gyq[q, s]
            gyt = const_pool.tile([128, CS], F32, tag=f"gy{i}")
            gy3 = gyt[:Q, :CS].rearrange("q (c s) -> q c s", c=nch, s=S)
            nc.gpsimd.iota(
                gy3, [[Q * S // nx, nch], [0, S]], base=0,
                channel_multiplier=0,
                allow_small_or_imprecise_dtypes=True,
            )
            nc.vector.tensor_tensor(
                gy3, gy3,
                gyq.unsqueeze(1).broadcast_to((Q, nch, S)), op=OP.add,
            )
            bt = const_pool.tile([128, CS * 12], BF16, tag=f"btl12{i}")
            btv = bt[:Q].rearrange("q (c s o) -> q c s o", c=nch, s=S, o=12)
            nc.scalar.mul(
                btv[:, :, :, 0:12:2],
                gxq.unsqueeze(1).unsqueeze(3).broadcast_to((Q, nch, S, 6)),
                stride,
            )
            nc.scalar.mul(
                btv[:, :, :, 1:12:2],
                gy3.unsqueeze(3).broadcast_to((Q, nch, S, 6)),
                stride,
            )
            ba = const_pool.tile([128, CS * 2], BF16, tag=f"btab{i}")
            bav = ba[:Q].rearrange("q (c s o) -> q c s o", c=nch, s=S, o=2)
            nc.vector.tensor_scalar(
                bav, btv[:, :, :, 0:2], -0.5 * stride, None, op0=OP.add
            )
            bt3 = const_pool.tile([128, NA * CS * 12], BF16, tag=f"btl3{i}")
            bt3v = bt3[:Q].rearrange(
                "q (a c s o) -> q a c s o", a=NA, c=nch, s=S, o=12
            )
            for a in range(NA):
                blr = wp[:Q, BLM_OFF + 36 * i + 12 * a:BLM_OFF + 36 * i + 12 * a + 12]
                nc.vector.tensor_tensor(
                    bt3v[:, a], btv,
                    blr.unsqueeze(1).unsqueeze(2)
                    .broadcast_to((Q, nch, S, 12)),
                    op=OP.add,
                )
            btl3_sb.append(bt3v)
            btab_sb.append(bav)

        wt_sb = []  # [scale][kc] -> [128, 57] AP
        off = 0
        for i in range(3):
            chunks = []
            for k in range(SCALES[i]["kc"]):
                chunks.append(wp[:, off:off + NCH])
                off += NCH
            wt_sb.append(chunks)
        b8_sb = [
            brt[32 * i:32 * i + 1, :].rearrange("p (sl c) -> p sl c", c=64)
            for i in range(3)
        ]
        a4_sb = [a4t[:, 6 * i:6 * i + 6] for i in range(3)]
        ones_sb = [wp[32 * i:32 * i + 1, W_ONES:W_ONES + 128] for i in range(3)]

        out_ap = out.ap()
        pending = []  # deferred per-chunk fixup+store emitters

        def do_chunk(si, b, x_aps, ch):
            """Emit one Q*S-pixel chunk: 32-slot psum groups + decode + store.

            x_aps: per-K-chunk [128, Q, S] SBUF APs (c, q, s).
            """
            s = SCALES[si]
            Q, S, kc = s["Q"], s["S"], s["kc"]
            stride = STRIDES[si]

            ot = o_pool.tile([128, 3 * 40 * NO], BF16)
            otv = ot[:Q, : NA * S * NO]
            o_v = otv.rearrange("q (a s o) -> q a s o", a=NA, s=S, o=NO)

            for g0 in range(0, S, GSL):
                gsl = min(GSL, S - g0)
                ps = ps_pool.tile([128, GSL * 64], F32)
                psv = ps[:Q]
                for sl in range(gsl):
                    for k in range(kc):
                        nc.tensor.matmul(
                            psv[:, sl * 64:sl * 64 + NCH],
                            lhsT=x_aps[k][:, :, g0 + sl],
                            rhs=wt_sb[si][k],
                            start=(sl * 64 % 512 == 0 and k == 0),
                            stop=False,
                        )
                p_sl = psv[:, : gsl * 64].rearrange("q (sl c) -> q sl c", c=64)
                # conv bias for the 21 sigmoid/cls channels of every slot
                # (lm bias rides the btl3 table); one matmul per bank
                # (matmul N is capped at one psum bank)
                for s0_ in range(0, gsl, 8):
                    s1_ = min(gsl, s0_ + 8)
                    nc.tensor.matmul(
                        p_sl[:, s0_:s1_, 0:21],
                        lhsT=ones_sb[si][:, :Q],
                        rhs=b8_sb[si][:, s0_:s1_, 0:21],
                        start=False,
                        stop=(s1_ == gsl),
                    )
                # sigmoid of o 0:5 (xy/wh/conf) straight into the output
                # tile; xy/wh are fixed up in place chunk-wide below
                nc.scalar.activation(
                    o_v[:, :, g0:g0 + gsl, 0:5],
                    p_sl[:, :, 0:15].rearrange(
                        "q sl (o a) -> q a sl o", o=5, a=NA
                    ),
                    AF.Sigmoid,
                )
                # cls: sigmoid straight into the output tile
                nc.scalar.activation(
                    o_v[:, :, g0:g0 + gsl, 17:19],
                    p_sl[:, :, 15:21].rearrange(
                        "q sl (o a) -> q a sl o", o=2, a=NA
                    ),
                    AF.Sigmoid,
                )
                # lm = p (anchor-scaled in weights) + grid + anchor bias
                btl = btl3_sb[si][:, :, ch, g0:g0 + gsl, :]
                nc.vector.tensor_tensor(
                    o_v[:, :, g0:g0 + gsl, 5:17],
                    p_sl[:, :, 21:NCH].rearrange(
                        "q sl (o a) -> q a sl o", o=12, a=NA
                    ),
                    btl, op=OP.add,
                )

            # ---- chunk-wide fixups + store, deferred one chunk -----------
            # (emitted after the NEXT chunk's groups, so the DVE queue ahead
            # of each psum-freeing lm add holds only lm adds and the PE
            # never stalls on a psum buffer)
            def finish(si=si, b=b, ch=ch, o_v=o_v, Q=Q, S=S, stride=stride):
                s = SCALES[si]
                # xy = sig*(2*stride) + btab, anchor-broadcast grid table
                btc = (
                    btab_sb[si][:, ch]
                    .unsqueeze(1)
                    .broadcast_to((Q, NA, S, 2))
                )
                nc.vector.scalar_tensor_tensor(
                    o_v[:, :, :, 0:2], o_v[:, :, :, 0:2], 2.0 * stride, btc,
                    op0=OP.mult, op1=OP.add,
                )
                # wh = square(sig) * 4*anchor: square on ACT, mult on gpsimd
                nc.scalar.square(o_v[:, :, :, 2:4], o_v[:, :, :, 2:4])
                a4 = (
                    a4_sb[si][:Q, :]
                    .rearrange("q (a o) -> q a o", a=NA, o=2)
                    .unsqueeze(2)
                    .broadcast_to((Q, NA, S, 2))
                )
                nc.gpsimd.tensor_tensor(
                    o_v[:, :, :, 2:4], o_v[:, :, :, 2:4], a4, op=OP.mult
                )
                # one store per chunk: S*38B contiguous per (q, anchor)
                dst = (
                    out_ap[b, OUT_BASE[si]:OUT_BASE[si] + NA * s["npix"], :]
                    .rearrange(
                        "(a ch q s) o -> ch q a s o",
                        a=NA, ch=s["nch"], q=Q, s=S,
                    )
                )
                nc.gpsimd.dma_start(dst[ch], o_v)

            pending.append(finish)
            while len(pending) > 1:
                pending.pop(0)()

        for b in range(dbg_imgs):
            if 0 in dbg_scales:
                s = SCALES[0]
                x0_flat = x_in[0].ap()[b].rearrange("c h w -> c (h w)")
                cpx = s["Q"] * s["S"]
                for ch in range(s["nch"]):
                    xt = x0_pool.tile([128, cpx], F8E3)
                    nc.sync.dma_start(
                        xt[:], x0_flat[:, ch * cpx:(ch + 1) * cpx]
                    )
                    x4 = xt[:].rearrange("c (s q) -> c q s", s=s["S"], q=s["Q"])
                    do_chunk(0, b, [x4], ch)

            if 1 in dbg_scales:
                s = SCALES[1]
                kc = s["kc"]
                x1_k = x_in[1].ap()[b].rearrange(
                    "(k c) h w -> c k (h w)", k=kc
                )
                cpx = s["Q"] * s["S"]
                for ch in range(s["nch"]):
                    t = x1_pool.tile([128, kc * cpx], F8E3)
                    nc.sync.dma_start(
                        t[:].rearrange("c (k p) -> c k p", k=kc),
                        x1_k[:, :, ch * cpx:(ch + 1) * cpx],
                    )
                    x5 = t[:].rearrange(
                        "c (k s q) -> c k q s", k=kc, s=s["S"], q=s["Q"]
                    )
                    do_chunk(1, b, [x5[:, k] for k in range(kc)], ch)

            if 2 in dbg_scales:
                s = SCALES[2]
                kc = s["kc"]
                x2_k = x_in[2].ap()[b].rearrange(
                    "(k c) h w -> c k (h w)", k=kc
                )
                t = x2_pool.tile([128, kc * s["npix"]], BF16)
                nc.sync.dma_start(
                    t[:].rearrange("c (k p) -> c k p", k=kc), x2_k
                )
                x5 = t[:].rearrange(
                    "c (k s q) -> c k q s", k=kc, s=s["S"], q=s["Q"]
                )
                do_chunk(2, b, [x5[:, k] for k in range(kc)], 0)

        while pending:
            pending.pop(0)()

    return nc


# Instruction types walrus accepts multiple sync-waits on.  Empirically none:
# even the kernel-tail Drain gets rejected with >1 wait.
_MULTI_WAIT_OK = set()


def _legalize_waits(nc):
    """Spill extra sync waits onto single-wait NoOps.

    walrus's per-instruction ISA structs hold a limited number of sync wait
    commands (a Matmult's LDWEIGHTS holds exactly one), and Tile's semaphore
    assignment doesn't know that.  Rewrite the scheduled program so every
    instruction carries at most one wait; the rest go to same-engine NoOps
    placed immediately before it (same blocking semantics).
    """
    f = nc.m.functions[0]
    for blk in f.blocks:
        insts = blk.instructions
        out = []
        changed = False
        for inst in insts:
            si = inst.sync_info
            if (
                si is not None
                and len(si.on_wait) > 1
                and type(inst).__name__ not in _MULTI_WAIT_OK
            ):
                waits = list(si.on_wait)
                for w in waits[:-1]:
                    nop = mybir.InstNoOp(
                        name=nc.get_next_instruction_name(),
                        engine=inst.engine,
                        ins=[],
                        outs=[],
                        sync_info=mybir.SyncInfo(on_wait=[w], on_update=[]),
                    )
                    out.append(nop)
                inst.sync_info = mybir.SyncInfo(
                    on_wait=[waits[-1]], on_update=list(si.on_update)
                )
                changed = True
            out.append(inst)
        if changed:
            blk.instructions = out


_NC_CACHE = None
_LEGALIZED = False


def _get_program(legalize=False):
    """Build (and cache) the Bass program.

    legalize=True applies the walrus wait-limit rewrite; the CoreSim can only
    run the raw (unlegalized) program, so this is done lazily for HW runs.
    """
    global _NC_CACHE, _LEGALIZED
    if _NC_CACHE is None:
        _NC_CACHE = _build_program()
    if legalize and not _LEGALIZED:
        _legalize_waits(_NC_CACHE)
        _LEGALIZED = True
    return _NC_CACHE


def _prep_inputs(x0, x1, x2, w0, w1, w2, b0, b1, b2):
    ws = (w0, w1, w2)
    bs = (b0, b1, b2)
    wpack = np.zeros((128, WP_W), dtype=ml_dtypes.bfloat16)
    brow = np.zeros((3, 32 * 64), dtype=ml_dtypes.bfloat16)
    off = 0
    for i in range(3):
        fac = _lm_factor(i)
        wt = (np.asarray(ws[i], np.float32).T * fac[None, :]).astype(np.float32)
        wt = wt[:, PERM]
        for k in range(SCALES[i]["kc"]):
            wpack[:, off:off + NCH] = wt[k * 128:(k + 1) * 128]
            off += NCH
        wpack[32 * i, W_ONES:W_ONES + 128] = 1.0
        bfac = np.asarray(bs[i], np.float32) * fac
        b57 = bfac[PERM]
        slot = np.concatenate([b57, np.zeros(64 - NCH, np.float32)])
        brow[i] = np.tile(slot, 32)
        blm = np.stack(
            [bfac[a * NO + 5:a * NO + 17] for a in range(NA)]
        ).reshape(-1)
        wpack[:, BLM_OFF + 36 * i:BLM_OFF + 36 * i + 36] = blm[None, :]
    x_np_dt = (ml_dtypes.float8_e3m4, ml_dtypes.float8_e3m4, ml_dtypes.bfloat16)
    xs = []
    for i, x in enumerate((x0, x1, x2)):
        sc = SCALES[i]
        v = np.asarray(x, np.float32).astype(x_np_dt[i])
        B, C = v.shape[0], v.shape[1]
        # (q, s) -> (s, q) within each chunk so matmul weight columns are
        # contiguous in SBUF (enables fast weight load on the PE)
        v = v.reshape(B, C, sc["nch"], sc["Q"], sc["S"])
        v = np.ascontiguousarray(v.transpose(0, 1, 2, 4, 3))
        xs.append(v.reshape(B, C, x.shape[2], x.shape[3]))
    in_maps = []
    for c in range(N_CORES):
        m = {"wpack": wpack, "brow": brow}
        for i, x in enumerate(xs):
            m[f"x{i}"] = np.ascontiguousarray(x[c * B_LOC:(c + 1) * B_LOC])
        in_maps.append(m)
    return in_maps


def _run(inputs, trace=False):
    nc = _get_program(legalize=True)
    in_maps = _prep_inputs(**inputs)
    res = run_bass_kernel_spmd(nc, in_maps, list(range(N_CORES)), trace=trace)
    out = np.concatenate([r["out"] for r in res.results], axis=0)
    return out.astype(np.float32), res


def kernel(x0, x1, x2, w0, w1, w2, b0, b1, b2):
    out, _ = _run(
        dict(x0=x0, x1=x1, x2=x2, w0=w0, w1=w1, w2=w2, b0=b0, b1=b1, b2=b2)
    )
    return out
